# revision 1
# baseline (speedup 1.0000x reference)
"""CRF loss (negative log-likelihood) on 8 TRN2 NeuronCores.

Strategy: pure data-parallel. The 1024-row batch is sharded 128 rows per
core; the tiny [64,64] transition matrix is replicated. Each core computes
two partial sums over its shard — sum_b forward[b] (log-partition scores)
and the gold-path score total — and the host combines:
    loss = (sum fwd - sum gold) / 1024.

Per-core kernel (B=128 batch, K=64 tags, T=512):

Forward scores — exp-domain recurrence in a tag-on-partition layout:
    p_t[i,b] = F_t[i,b] * sum_j Et[j,i] p_{t-1}[j,b]
    Et[j,i] = exp(transitions[i,j])/128, F_t[i,b] = exp(feats[b,t,i])
one [64x64]@[64x128] matmul (PE) + one elementwise multiply (DVE) per
step, with periodic per-column renormalization (column sums via a
ones-matmul; log of the normalizer accumulates into clog).
F_t is produced by a bulk exp on ScalarE in natural layout followed by a
matmul-with-identity transpose on the PE.

Gold score — the loss only needs global sums, so gathers become matmuls:
one-hot masks MT_t[b,i] = (tags[b,t]==i) are built in natural layout on
DVE (iota compare), and a single PSUM-accumulated matmul per step forms
    N[i,j]  = sum_{b,t} MT_t[b,i] MT_{t-1}[b,j]   (transition pair counts)
    E2[i,k] = sum_{b,t} MT_t[b,i] feats_t[b,k]    (emit matrix)
whence gold_total = <N, transitions> + trace(E2), exact in f32.
"""
import sys
sys.path.insert(0, "/opt/trn_rl_repo")
import contextlib
import numpy as np
import ml_dtypes

import concourse.bass as bass
import concourse.mybir as mybir
from concourse.tile import TileContext
from concourse.bass_utils import run_bass_kernel_spmd

# antenv.axon_hooks is absent in this container; bass_utils only needs it
# for the optional NTFF-trace path and handles a None hook gracefully.
try:
    import antenv.axon_hooks  # noqa: F401
except ImportError:
    import types as _types
    import antenv as _antenv
    _m = _types.ModuleType("antenv.axon_hooks")
    _m.get_axon_ntff_profile_hook = lambda: None
    sys.modules["antenv.axon_hooks"] = _m
    _antenv.axon_hooks = _m

F32 = mybir.dt.float32
BF16 = mybir.dt.bfloat16
AF = mybir.ActivationFunctionType
OP = mybir.AluOpType

K = 64
B = 128            # batch rows per core
NCORES = 8
START = 62
LOG128 = float(np.log(128.0))

# ---------------------------------------------------------------------------
# Workarounds for this container's walrus build: each instruction may carry
# at most ONE sync-wait command (two for EventSemaphore). TileContext's exit
# barrier and scheduler can emit more; hoist extras onto NoOps.
# ---------------------------------------------------------------------------
from concourse import tile as tile_mod
from bass_rust import ScopedClock


def _drain_and_barrier_split(self, tick_clock, wait_clock):
    probe = self.nc.sync.nop(nofuse=True, hint="tile_exit_waits")
    wait_clock.add_sem_waits(
        probe.ins, ScopedClock({None: tick_clock.global_clock}))
    si = probe.ins.sync_info
    waits = list(si.on_wait) if si is not None and si.on_wait else []
    if len(waits) > 1:
        probe.ins.sync_info = mybir.SyncInfo(on_wait=[waits[0]], on_update=[])
        for w in waits[1:]:
            nop = self.nc.sync.nop(nofuse=True, hint="tile_exit_waits")
            nop.ins.sync_info = mybir.SyncInfo(on_wait=[w], on_update=[])
    self.nc.sync.drain()
    self.nc.all_engine_barrier()
    assert self.sems is not None
    popped = self.nc._tile_sem_poison_stack.pop()
    assert popped is self._sem_poison
    self.nc.clear_and_free_semaphores(list(self.sems.allocated().values()))
    self.nc.all_engine_barrier()


tile_mod.TileContext._drain_and_barrier = _drain_and_barrier_split


def _split_excess_waits(nc):
    n_split = 0
    for f in nc.m.functions:
        for blk in f.blocks:
            insts = blk.instructions
            new_insts = []
            for inst in insts:
                si = inst.sync_info
                cap = 2 if type(inst).__name__ == "InstEventSemaphore" else 1
                if si is not None and si.on_wait and len(si.on_wait) > cap:
                    waits = list(si.on_wait)
                    keep = waits[: cap - 1] if cap > 1 else []
                    spill = waits[len(keep): -1]
                    last = waits[-1]
                    for w in spill:
                        n_split += 1
                        nop = mybir.InstNoOp(
                            name=f"{inst.name}-waitsplit{n_split}",
                            ins=[], outs=[])
                        nop.engine = inst.engine
                        nop.sync_info = mybir.SyncInfo(on_wait=[w], on_update=[])
                        new_insts.append(nop)
                    inst.sync_info = mybir.SyncInfo(
                        on_wait=keep + [last],
                        on_update=list(si.on_update) if si.on_update else [])
                new_insts.append(inst)
            if len(new_insts) != len(insts):
                blk.instructions = new_insts
    return n_split


# ---------------------------------------------------------------------------
# Kernel builder
# ---------------------------------------------------------------------------
def build_crf(T=512, R=16, chunk=64):
    TS = T - 1
    nchunks = (TS + chunk - 1) // chunk

    nc = bass.Bass()
    feats = nc.dram_tensor("feats", [B, T, K], F32, kind="ExternalInput")
    tags = nc.dram_tensor("tags", [B, T], mybir.dt.int32, kind="ExternalInput")
    trans = nc.dram_tensor("trans", [K, K], F32, kind="ExternalInput")
    out = nc.dram_tensor("out", [1, 2], F32, kind="ExternalOutput")

    eye64_f = nc.inline_tensor(
        np.ascontiguousarray(np.eye(K, dtype=np.float32)), name="eye64f")
    iota_row = nc.inline_tensor(
        np.ascontiguousarray(
            np.broadcast_to(np.arange(K, dtype=np.float32), (B, K))
            .astype(ml_dtypes.bfloat16)),
        name="iota_row")
    ones_k1_bf = nc.inline_tensor(
        np.ascontiguousarray(np.ones((K, 1), np.float32).astype(ml_dtypes.bfloat16)),
        name="ones_k1_bf")
    ones_1k_f = nc.inline_tensor(
        np.ascontiguousarray(np.ones((1, K), np.float32)), name="ones_1k_f")
    ones_k1_f = nc.inline_tensor(
        np.ascontiguousarray(np.ones((K, 1), np.float32)), name="ones_k1_f")
    i128_bf = nc.inline_tensor(
        np.ascontiguousarray(np.eye(B, dtype=np.float32).astype(ml_dtypes.bfloat16)),
        name="i128bf")
    p0_np = np.zeros((K, B), np.float32)
    p0_np[START, :] = 1.0
    p0_dram = nc.inline_tensor(
        np.ascontiguousarray(p0_np.astype(ml_dtypes.bfloat16)), name="p0")

    with TileContext(nc) as tc:
        with contextlib.ExitStack() as ctx:
            consts = ctx.enter_context(tc.tile_pool(name="consts", bufs=1))
            feats_pool = ctx.enter_context(tc.tile_pool(name="feats", bufs=2))
            fnat_pool = ctx.enter_context(tc.tile_pool(name="fnat", bufs=2))
            c_pool = ctx.enter_context(tc.tile_pool(name="cpool", bufs=2))
            ft_pool = ctx.enter_context(tc.tile_pool(name="ftp", bufs=2))
            small = ctx.enter_context(tc.tile_pool(name="small", bufs=2))
            p_pool = ctx.enter_context(tc.tile_pool(name="ppool", bufs=3))
            psum_q = ctx.enter_context(tc.tile_pool(name="psq", bufs=2, space="PSUM"))
            psum_t = ctx.enter_context(tc.tile_pool(name="pst", bufs=2, space="PSUM"))
            psum_g = ctx.enter_context(tc.tile_pool(name="psg", bufs=1, space="PSUM"))
            psum_s = ctx.enter_context(tc.tile_pool(name="pss", bufs=2, space="PSUM"))

            eye_f = consts.tile([K, K], F32)
            nc.sync.dma_start(eye_f[:], eye64_f[:])
            iota_sb = consts.tile([B, K], BF16)
            nc.sync.dma_start(iota_sb[:], iota_row[:])
            ones_k1 = consts.tile([K, 1], BF16)
            nc.sync.dma_start(ones_k1[:], ones_k1_bf[:])
            ones_1k = consts.tile([1, K], F32)
            nc.sync.dma_start(ones_1k[:], ones_1k_f[:])
            ones_kf = consts.tile([K, 1], F32)
            nc.sync.dma_start(ones_kf[:], ones_k1_f[:])
            tr_sb = consts.tile([K, K], F32)
            nc.sync.dma_start(tr_sb[:], trans[:])
            i128 = consts.tile([B, B], BF16)
            nc.sync.dma_start(i128[:], i128_bf[:])

            # Et[j,i] = exp(trans[i,j] - log 128)  (bf16)
            trT_ps = psum_s.tile([K, K], F32, tag="misc")
            nc.tensor.transpose(trT_ps[:], tr_sb[:], eye_f[:])
            Et = consts.tile([K, K], BF16)
            nlog128 = consts.tile([K, 1], F32)
            nc.vector.memset(nlog128[:], -LOG128)
            nc.scalar.activation(Et[:], trT_ps[:], AF.Exp, bias=nlog128[:])

            # gold multiplier: cols 0:64 = trans, 64:128 = eye (f32)
            gmult = consts.tile([K, 2 * K], F32)
            nc.vector.tensor_copy(gmult[:, 0:K], tr_sb[:])
            nc.vector.tensor_copy(gmult[:, K:2 * K], eye_f[:])

            tags_sb = consts.tile([B, T], mybir.dt.int32)
            nc.sync.dma_start(tags_sb[:], tags[:])
            tags_bf = consts.tile([B, T], BF16)
            nc.vector.tensor_copy(tags_bf[:], tags_sb[:])

            p_t = p_pool.tile([K, B], BF16, tag="p")
            nc.sync.dma_start(p_t[:], p0_dram[:])
            clog = consts.tile([1, B], F32)
            nc.vector.memset(clog[:], TS * LOG128)

            NE2 = psum_g.tile([K, 2 * K], F32)

            for c in range(nchunks):
                base = c * chunk
                nstep = min(chunk, TS - base)
                nmask = min(chunk + 1, T - base)

                # C tile: slot s holds [MT_{base+s} | feats_{base+1+s}]
                ct = c_pool.tile([B, chunk + 1, 2 * K], BF16, tag="C")
                tb = tags_bf[:]
                tags_view = bass.AP(
                    tensor=tb.tensor, offset=tb.offset + base,
                    ap=[tb.ap[0], [1, nmask], [0, K]])
                io = iota_sb[:]
                iota_view = bass.AP(
                    tensor=io.tensor, offset=io.offset,
                    ap=[io.ap[0], [0, nmask], [1, K]])
                nc.vector.tensor_tensor(
                    ct[:, 0:nmask, 0:K], tags_view, iota_view, OP.is_equal)

                fch = feats_pool.tile([B, chunk, K], F32, tag="fch")
                nc.sync.dma_start(
                    fch[:, 0:nstep, :], feats[:, base + 1: base + 1 + nstep, :])
                nc.vector.tensor_scalar(
                    ct[:, 0:nstep, K:2 * K], fch[:, 0:nstep, :], 1.0, None,
                    OP.mult)
                fnat = fnat_pool.tile([B, chunk, K], BF16, tag="fnat")
                nc.scalar.activation(fnat[:, 0:nstep, :], fch[:, 0:nstep, :],
                                     AF.Exp)

                # F transposed via matmul-with-identity (4 slots per PSUM tile)
                ftc = ft_pool.tile([K, chunk, B], BF16, tag="ftc")
                for g in range(0, nstep, 4):
                    ng = min(4, nstep - g)
                    tp = psum_t.tile([K, 4 * B], F32, tag="tp")
                    for s in range(g, g + ng):
                        col = (s - g) * B
                        nc.tensor.matmul(
                            tp[:, col:col + B], fnat[:, s, :], i128[:],
                            start=True, stop=True)
                    nc.scalar.copy(
                        ftc[:, g:g + ng, :].rearrange("p a b -> p (a b)"),
                        tp[:, 0:ng * B])

                for s in range(nstep):
                    t = base + 1 + s
                    qp = psum_q.tile([K, B], F32, tag="q")
                    nc.tensor.matmul(qp[:], Et[:], p_t[:], start=True, stop=True)
                    p_new = p_pool.tile([K, B], BF16, tag="p")
                    nc.vector.tensor_tensor(
                        p_new[:], qp[:], ftc[:, s, :], OP.mult)
                    p_t = p_new

                    nc.tensor.matmul(
                        NE2[:], ct[:, s + 1, 0:K], ct[:, s, :],
                        start=(t == 1), stop=(t == TS))

                    if t % R == 0 or t == TS:
                        sp = psum_s.tile([1, B], F32, tag="misc")
                        nc.tensor.matmul(sp[:], ones_k1[:], p_t[:],
                                         start=True, stop=True)
                        logS = small.tile([1, B], F32, tag="logS")
                        nc.scalar.activation(logS[:], sp[:], AF.Ln)
                        nc.vector.tensor_tensor(clog[:], clog[:], logS[:], OP.add)
                        if t != TS:
                            rec = small.tile([1, B], F32, tag="rec")
                            nc.vector.reciprocal(rec[:], sp[:])
                            rb = psum_s.tile([K, B], F32, tag="misc")
                            nc.tensor.matmul(rb[:], ones_1k[:], rec[:],
                                             start=True, stop=True)
                            p_sc = p_pool.tile([K, B], BF16, tag="p")
                            nc.vector.tensor_tensor(p_sc[:], rb[:], p_t[:],
                                                    OP.mult)
                            p_t = p_sc

            # epilogue
            fsum = small.tile([1, 1], F32, tag="fsum")
            nc.vector.tensor_reduce(
                fsum[:], clog[:], axis=mybir.AxisListType.X, op=OP.add)
            gs = small.tile([K, 2 * K], F32, tag="gs")
            nc.scalar.copy(gs[:], NE2[:])
            gw = small.tile([K, 2 * K], F32, tag="gw")
            nc.vector.tensor_tensor(gw[:], gs[:], gmult[:], OP.mult)
            gr = small.tile([K, 1], F32, tag="gr")
            nc.vector.tensor_reduce(
                gr[:], gw[:], axis=mybir.AxisListType.X, op=OP.add)
            gsum_ps = psum_s.tile([1, 1], F32, tag="misc")
            nc.tensor.matmul(gsum_ps[:], ones_kf[:], gr[:], start=True, stop=True)

            outt = small.tile([1, 2], F32, tag="outt")
            nc.vector.tensor_copy(outt[:, 0:1], fsum[:])
            nc.scalar.copy(outt[:, 1:2], gsum_ps[:])
            nc.sync.dma_start(out[:], outt[:])

    _split_excess_waits(nc)
    return nc


_cached = {}


def _get_nc(T):
    if T not in _cached:
        _cached[T] = build_crf(T=T)
    return _cached[T]


def kernel(feats, tags, transitions, _trace=False):
    feats = np.ascontiguousarray(np.asarray(feats, dtype=np.float32))
    tags = np.ascontiguousarray(np.asarray(tags).astype(np.int32))
    transitions = np.ascontiguousarray(np.asarray(transitions, dtype=np.float32))
    Btot, T, k = feats.shape
    assert k == K and Btot % NCORES == 0
    bs = Btot // NCORES
    assert bs == B, f"kernel hardcodes {B} rows/core, got {bs}"

    nc = _get_nc(T)
    in_maps = [
        {"feats": feats[i * B:(i + 1) * B],
         "tags": tags[i * B:(i + 1) * B],
         "trans": transitions}
        for i in range(NCORES)
    ]
    res = run_bass_kernel_spmd(nc, in_maps, core_ids=list(range(NCORES)),
                               trace=_trace)
    fwd = 0.0
    gold = 0.0
    for r in res.results:
        fwd += float(r["out"][0, 0])
        gold += float(r["out"][0, 1])
    loss = np.float32((fwd - gold) / Btot)
    if _trace:
        return np.asarray(loss), res
    return np.asarray(loss)

